# revision 2
# baseline (speedup 1.0000x reference)
"""Multi-head attention (B=2, L=2048, DM=1024, H=16, dk=dv=64) on 8 TRN2 cores.

Sharding: core c handles batch b = c//4 and heads [4g, 4g+4), g = c%4
(data parallel on B x tensor parallel on heads). Returns (context, attention)
matching the reference. The output projection is row-parallel: each core
produces a partial (2048, 1024) product; the host sums the 4 partials per
batch (cheap) instead of an on-device all-reduce.

Device pipeline per core (all in "S-transposed" orientation, which makes
every matmul contraction land on the partition axis with zero on-device
transposes):
  proj:   qhT/khT [d, L] (f32r), vh [j, d] (fp16)
  S^T     [j, i] = khT.T @ qhT   (f32r matmul, K=64)
  expS    = exp(S^T * 1/sqrt(dk))   (ScalarE, PSUM->SBUF, fp16 out)
  mexpS   = expS * maskT01          (VectorE, fp16)
  denom   = ones.T @ mexpS          (PE partition-reduction)
  recipB  = broadcast(1/denom)      (PE ones-outer-product)
  P^T     = mexpS * recipB in-place (GpSimd)  -> attnT output tiles
  ctxT    [d, i] += vh.T @ P^T      (PE, fp16)
  out     [i, n] = ctxT.T @ woT     (PE, fp16 -> f32 psum)
Host transposes the attention output back to natural [i, j] layout.
"""

import numpy as np
import ml_dtypes

import concourse.bass as bass
import concourse.bacc as bacc
import concourse.mybir as mybir
import concourse.tile as tile
from concourse.bass_utils import run_bass_kernel_spmd

# ---- problem constants (hardcoded per contract) ----
B, L, DM, NHEAD, DK, DV = 2, 2048, 1024, 16, 64, 64
NC_ = 8                 # cores
GPB = 4                 # head-groups (cores) per batch
NH = NHEAD // GPB       # heads per core = 4
DHB = NH * DK           # 256: d-block per core
P = 128
NJT = L // P            # 16 j tiles
NIC = 4                 # i chunks of 512
IC = L // NIC           # 512
NMT = DM // P           # 8 m tiles
SCALE = 1.0 / np.sqrt(DK)

f32 = mybir.dt.float32
f32r = mybir.dt.float32r
fp16 = mybir.dt.float16

_CACHE = {}


def _round_fp32r(x):
    u = np.ascontiguousarray(np.asarray(x, np.float32)).view(np.uint32)
    u2 = u + np.uint32(0x7FF) + ((u >> np.uint32(12)) & np.uint32(1))
    return (u2 & np.uint32(0xFFFFF000)).view(np.float32)


def _build():
    nc = bacc.Bacc("TRN2", target_bir_lowering=False, debug=False,
                   num_devices=NC_)

    qT = nc.dram_tensor("qT", [DM, L], f32r, kind="ExternalInput").ap()
    kT = nc.dram_tensor("kT", [DM, L], f32r, kind="ExternalInput").ap()
    vT = nc.dram_tensor("vT", [DM, L], f32r, kind="ExternalInput").ap()
    maskT = nc.dram_tensor("maskT", [L, L], fp16, kind="ExternalInput").ap()
    wqT = nc.dram_tensor("wqT", [DM, DHB], f32r, kind="ExternalInput").ap()
    wkT = nc.dram_tensor("wkT", [DM, DHB], f32r, kind="ExternalInput").ap()
    wvT = nc.dram_tensor("wvT", [DM, DHB], f32r, kind="ExternalInput").ap()
    woT = nc.dram_tensor("woT", [DHB, DM], fp16, kind="ExternalInput").ap()

    attnT = nc.dram_tensor("attnT", [NH, L, L], fp16, kind="ExternalOutput").ap()
    outp = nc.dram_tensor("outp", [L, DM], f32, kind="ExternalOutput").ap()

    # tiled HBM views
    qT_t = qT.rearrange("(t p) i -> p t i", p=P)      # [128, 8, 2048]
    kT_t = kT.rearrange("(t p) i -> p t i", p=P)
    vT_t = vT.rearrange("(t p) i -> p t i", p=P)
    wqT_t = wqT.rearrange("(t p) d -> p t d", p=P)    # [128, 8, 256]
    wkT_t = wkT.rearrange("(t p) d -> p t d", p=P)
    wvT_t = wvT.rearrange("(t p) d -> p t d", p=P)
    woT_t = woT.rearrange("(t p) n -> p t n", p=P)    # [128, 2, 1024]
    maskT_t = maskT.rearrange("(t p) i -> p t i", p=P)  # [128, 16, 2048]
    attnT_t = attnT.rearrange("h (t p) i -> p h t i", p=P)  # [128,4,16,2048]
    outp_t = outp.rearrange("(t p) n -> p t n", p=P)  # [128, 16, 1024]

    # persistent SBUF
    qhT = nc.alloc_sbuf_tensor("qhT", [P, 2, L], f32r).ap()
    khT = nc.alloc_sbuf_tensor("khT", [P, 2, L], f32r).ap()
    vh = nc.alloc_sbuf_tensor("vh", [P, NJT, DHB], fp16).ap()
    ctxT = nc.alloc_sbuf_tensor("ctxT", [P, 2, L], fp16).ap()
    woT_s = nc.alloc_sbuf_tensor("woT_s", [P, 2, DM], fp16).ap()
    ones_col = nc.alloc_sbuf_tensor("ones_col", [P, 1], fp16).ap()
    ones_row = nc.alloc_sbuf_tensor("ones_row", [1, P], fp16).ap()

    with tile.TileContext(nc) as tc:
        with (
            tc.tile_pool(name="xs", bufs=2) as xs_pool,
            tc.tile_pool(name="w", bufs=2) as w_pool,
            tc.tile_pool(name="mk", bufs=2) as mk_pool,
            tc.tile_pool(name="ex", bufs=3) as ex_pool,
            tc.tile_pool(name="st", bufs=2) as st_pool,
            tc.tile_pool(name="rb", bufs=2) as rb_pool,
            tc.tile_pool(name="ro", bufs=2) as ro_pool,
            tc.tile_pool(name="ou", bufs=2) as ou_pool,
            tc.tile_pool(name="ps", bufs=2, space="PSUM") as ps_pool,
            tc.tile_pool(name="aux", bufs=3, space="PSUM") as aux_pool,
        ):
            nc.vector.memset(ones_col[:, :], 1.0)
            nc.vector.memset(ones_row[:, :], 1.0)
            nc.sync.dma_start(out=woT_s[:, :, :], in_=woT_t)

            # ---------------- projections ----------------
            # q/k -> qhT/khT [128(d), dchunk, i] = wT.T @ xT, f32r
            for src_t, w_t, dst in (
                (qT_t, wqT_t, qhT),
                (kT_t, wkT_t, khT),
            ):
                w_s = w_pool.tile([P, NMT, DHB], f32r, tag="w")
                nc.sync.dma_start(out=w_s[:, :, :], in_=w_t)
                for ic in range(NIC):
                    x_s = xs_pool.tile([P, NMT, IC], f32r, tag="xs")
                    nc.sync.dma_start(
                        out=x_s[:, :, :],
                        in_=src_t[:, :, ic * IC:(ic + 1) * IC],
                    )
                    for dc in range(2):
                        pp = ps_pool.tile([P, IC], f32, tag="ps")
                        for m in range(NMT):
                            nc.tensor.matmul(
                                pp[:, :],
                                w_s[:, m, dc * P:(dc + 1) * P],
                                x_s[:, m, :],
                                start=(m == 0),
                                stop=(m == NMT - 1),
                            )
                        nc.scalar.copy(
                            dst[:, dc, ic * IC:(ic + 1) * IC], pp[:, :]
                        )

            # v -> vh [128(j), jt, d] = vT_slice.T @ wvT, fp16
            w_s = w_pool.tile([P, NMT, DHB], f32r, tag="w")
            nc.sync.dma_start(out=w_s[:, :, :], in_=wvT_t)
            for jt in range(NJT):
                x_s = xs_pool.tile([P, NMT, P], f32r, tag="xs")
                nc.sync.dma_start(
                    out=x_s[:, :, :], in_=vT_t[:, :, jt * P:(jt + 1) * P]
                )
                pp = ps_pool.tile([P, DHB], f32, tag="ps")
                for m in range(NMT):
                    nc.tensor.matmul(
                        pp[:, :],
                        x_s[:, m, :],
                        w_s[:, m, :],
                        start=(m == 0),
                        stop=(m == NMT - 1),
                    )
                nc.scalar.copy(vh[:, jt, :], pp[:, :])

            # ---------------- attention ----------------
            for ic in range(NIC):
                mk = mk_pool.tile([P, NJT, IC], fp16, tag="mk")
                nc.sync.dma_start(
                    out=mk[:, :, :],
                    in_=maskT_t[:, :, ic * IC:(ic + 1) * IC],
                )
                for h in range(NH):
                    po = 64 * (h % 2)      # partition offset for this head
                    dc = h // 2            # d-chunk holding this head
                    stripe = st_pool.tile([P, NJT, IC], fp16, tag="st")

                    # S^T tiles + exp + mask, 2 j-tiles at a time
                    for j2 in range(NJT // 2):
                        sp = ps_pool.tile([P, 2, IC], f32, tag="ps")
                        for u in range(2):
                            jt = 2 * j2 + u
                            nc.tensor.matmul(
                                sp[:, u, :],
                                khT[po:po + 64, dc, jt * P:(jt + 1) * P],
                                qhT[po:po + 64, dc, ic * IC:(ic + 1) * IC],
                                start=True,
                                stop=True,
                            )
                        ex = ex_pool.tile([P, 2, IC], fp16, tag="ex")
                        nc.scalar.activation(
                            ex[:, :, :],
                            sp[:, :, :],
                            mybir.ActivationFunctionType.Exp,
                            scale=float(SCALE),
                        )
                        nc.vector.tensor_tensor(
                            out=stripe[:, 2 * j2:2 * j2 + 2, :],
                            in0=ex[:, :, :],
                            in1=mk[:, 2 * j2:2 * j2 + 2, :],
                            op=mybir.AluOpType.mult,
                        )

                    # denominator: ones.T @ mexpS accumulated over j tiles
                    dn = aux_pool.tile([P, IC], f32, tag="aux")
                    for jt in range(NJT):
                        nc.tensor.matmul(
                            dn[:1, :],
                            ones_col[:, :],
                            stripe[:, jt, :],
                            start=(jt == 0),
                            stop=(jt == NJT - 1),
                        )
                    ro = ro_pool.tile([1, IC], fp16, tag="ro")
                    with nc.allow_low_precision("fp16 softmax denominators"):
                        nc.vector.reciprocal(ro[:1, :], dn[:1, :])
                    rbp = aux_pool.tile([P, IC], f32, tag="aux")
                    nc.tensor.matmul(
                        rbp[:, :], ones_row[:1, :], ro[:1, :],
                        start=True, stop=True,
                    )
                    rb = rb_pool.tile([P, IC], fp16, tag="rb")
                    nc.scalar.copy(rb[:, :], rbp[:, :])

                    # normalize in place -> P^T, then store + context
                    nc.gpsimd.tensor_tensor(
                        out=stripe[:, :, :],
                        in0=stripe[:, :, :],
                        in1=rb[:, None, :].broadcast_to([P, NJT, IC]),
                        op=mybir.AluOpType.mult,
                    )
                    nc.sync.dma_start(
                        out=attnT_t[:, h, :, ic * IC:(ic + 1) * IC],
                        in_=stripe[:, :, :],
                    )
                    cx = aux_pool.tile([P, IC], f32, tag="aux")
                    for jt in range(NJT):
                        nc.tensor.matmul(
                            cx[po:po + 64, :],
                            vh[:, jt, h * DK:(h + 1) * DK],
                            stripe[:, jt, :],
                            start=(jt == 0),
                            stop=(jt == NJT - 1),
                            tile_position=(0, po),
                        )
                    nc.scalar.copy(
                        ctxT[po:po + 64, dc, ic * IC:(ic + 1) * IC],
                        cx[po:po + 64, :],
                    )

            # ---------------- output projection ----------------
            for it in range(NJT):
                ob = ou_pool.tile([P, DM], f32, tag="ou")
                for nch in range(2):
                    op = aux_pool.tile([P, IC], f32, tag="aux")
                    for kt in range(2):
                        nc.tensor.matmul(
                            op[:, :],
                            ctxT[:, kt, it * P:(it + 1) * P],
                            woT_s[:, kt, nch * IC:(nch + 1) * IC],
                            start=(kt == 0),
                            stop=(kt == 1),
                        )
                    nc.vector.tensor_copy(
                        ob[:, nch * IC:(nch + 1) * IC], op[:, :]
                    )
                nc.sync.dma_start(out=outp_t[:, it, :], in_=ob[:, :])

    nc.compile()
    return nc


def get_nc():
    if "nc" not in _CACHE:
        _CACHE["nc"] = _build()
    return _CACHE["nc"]


def prep_in_maps(q, k, v, mask, wq, wk, wv, wo_w, wo_b):
    q = np.asarray(q, np.float32)
    k = np.asarray(k, np.float32)
    v = np.asarray(v, np.float32)
    mask = np.asarray(mask)

    maskT_h = [
        np.ascontiguousarray((mask[b_] != 0).T.astype(np.float16))
        for b_ in range(B)
    ]
    qT_h = [_round_fp32r(q[b_].T) for b_ in range(B)]
    kT_h = [_round_fp32r(k[b_].T) for b_ in range(B)]
    vT_h = [_round_fp32r(v[b_].T) for b_ in range(B)]

    in_maps = []
    for c in range(NC_):
        b_ = c // GPB
        g = c % GPB
        ds = slice(DHB * g, DHB * (g + 1))
        in_maps.append({
            "qT": qT_h[b_],
            "kT": kT_h[b_],
            "vT": vT_h[b_],
            "maskT": maskT_h[b_],
            "wqT": _round_fp32r(wq[ds].T),
            "wkT": _round_fp32r(wk[ds].T),
            "wvT": _round_fp32r(wv[ds].T),
            "woT": np.ascontiguousarray(wo_w[:, ds].T.astype(np.float16)),
        })
    return in_maps


def kernel(q, k, v, mask, wq, wk, wv, wo_w, wo_b):
    nc = get_nc()
    wo_b = np.asarray(wo_b, np.float32)
    in_maps = prep_in_maps(q, k, v, mask, wq, wk, wv, wo_w, wo_b)
    _CACHE["last_in_maps"] = in_maps

    res = run_bass_kernel_spmd(nc, in_maps, list(range(NC_)))
    _CACHE["last_results"] = res

    attention = np.empty((B, NHEAD, L, L), np.float32)
    context = np.zeros((B, L, DM), np.float32)
    for c in range(NC_):
        b_ = c // GPB
        g = c % GPB
        at = res.results[c]["attnT"]  # [NH, L(j), L(i)] fp16
        for hl in range(NH):
            attention[b_, NH * g + hl] = at[hl].T.astype(np.float32)
        context[b_] += res.results[c]["outp"]
    context += wo_b[None, None, :]
    return context, attention


# revision 3
# speedup vs baseline: 1.2739x; 1.2739x over previous
"""Multi-head attention (B=2, L=2048, DM=1024, H=16, dk=dv=64) on 8 TRN2 cores.

Sharding: core c handles batch b = c//4 and heads [4g, 4g+4), g = c%4
(data parallel on B x tensor parallel on heads). Returns (context, attention)
matching the reference. The output projection is row-parallel: each core
produces a partial (2048, 1024) product; the host sums the 4 partials per
batch instead of an on-device all-reduce.

Device pipeline per core, all in "S-transposed" orientation (every matmul
contraction lands on the partition axis; zero on-device transposes):
  proj:   qhT/khT [d, L], vh [j, d]            (fp16 matmuls, f32 psum)
  S^T     [j, i] = khT.T @ qhT                 (PE, K=64)
  expS    = exp(S^T / sqrt(dk))                (ScalarE, PSUM->SBUF fp16)
  mexpS   = expS * maskT01                     (VectorE)
  denom   = ones.T @ mexpS                     (PE partition-reduction)
  recipB  = 1 / broadcast(denom)               (PE ones-outer + VectorE recip)
  P^T     = mexpS * recipB in-place            (VectorE + GpSimd split)
  ctxT    [d, i] += vh.T @ P^T                 (PE)
  out     [i, n] = ctxT.T @ woT                (PE, f32 psum)
The (ic, h) steps are software-pipelined one deep: each step's context
matmuls are emitted after the NEXT step's S^T/denominator matmuls so the
tensor engine never stalls on the softmax chain (keeps HAM un-throttled).
Host transposes the attention output back to natural [i, j] layout.
"""

import numpy as np

import concourse.bass as bass
import concourse.bacc as bacc
import concourse.mybir as mybir
import concourse.tile as tile
from concourse.bass_utils import run_bass_kernel_spmd

# ---- problem constants (hardcoded per contract) ----
B, L, DM, NHEAD, DK, DV = 2, 2048, 1024, 16, 64, 64
NC_ = 8                 # cores
GPB = 4                 # head-groups (cores) per batch
NH = NHEAD // GPB       # heads per core = 4
DHB = NH * DK           # 256: d-block per core
P = 128
NJT = L // P            # 16 j tiles
NIC = 4                 # i chunks
IC = L // NIC           # 512
NMT = DM // P           # 8 m tiles
SCALE = 1.0 / np.sqrt(DK)
GJT = 12                # normalize: j-tiles 0..GJT on DVE, rest on GpSimd

f32 = mybir.dt.float32
fp16 = mybir.dt.float16

_CACHE = {}


def _build():
    nc = bacc.Bacc("TRN2", target_bir_lowering=False, debug=False,
                   num_devices=NC_)

    qT = nc.dram_tensor("qT", [DM, L], fp16, kind="ExternalInput").ap()
    kT = nc.dram_tensor("kT", [DM, L], fp16, kind="ExternalInput").ap()
    vT = nc.dram_tensor("vT", [DM, L], fp16, kind="ExternalInput").ap()
    maskT = nc.dram_tensor("maskT", [L, L], fp16, kind="ExternalInput").ap()
    wqT = nc.dram_tensor("wqT", [DM, DHB], fp16, kind="ExternalInput").ap()
    wkT = nc.dram_tensor("wkT", [DM, DHB], fp16, kind="ExternalInput").ap()
    wvT = nc.dram_tensor("wvT", [DM, DHB], fp16, kind="ExternalInput").ap()
    woT = nc.dram_tensor("woT", [DHB, DM], fp16, kind="ExternalInput").ap()

    attnT = nc.dram_tensor("attnT", [NH, L, L], fp16, kind="ExternalOutput").ap()
    outp = nc.dram_tensor("outp", [L, DM], f32, kind="ExternalOutput").ap()

    # tiled HBM views
    qT_t = qT.rearrange("(t p) i -> p t i", p=P)      # [128, 8, 2048]
    kT_t = kT.rearrange("(t p) i -> p t i", p=P)
    vT_t = vT.rearrange("(t p) i -> p t i", p=P)
    wqT_t = wqT.rearrange("(t p) d -> p t d", p=P)    # [128, 8, 256]
    wkT_t = wkT.rearrange("(t p) d -> p t d", p=P)
    wvT_t = wvT.rearrange("(t p) d -> p t d", p=P)
    woT_t = woT.rearrange("(t p) n -> p t n", p=P)    # [128, 2, 1024]
    maskT_t = maskT.rearrange("(t p) i -> p t i", p=P)  # [128, 16, 2048]
    attnT_t = attnT.rearrange("h (t p) i -> p h t i", p=P)  # [128,4,16,2048]
    outp_t = outp.rearrange("(t p) n -> p t n", p=P)  # [128, 16, 1024]

    # persistent SBUF
    qhT = nc.alloc_sbuf_tensor("qhT", [P, 2, L], fp16).ap()
    khT = nc.alloc_sbuf_tensor("khT", [P, 2, L], fp16).ap()
    vh = nc.alloc_sbuf_tensor("vh", [P, NJT, DHB], fp16).ap()
    ctxT = nc.alloc_sbuf_tensor("ctxT", [P, 2, L], fp16).ap()
    woT_s = nc.alloc_sbuf_tensor("woT_s", [P, 2, DM], fp16).ap()
    ones_col = nc.alloc_sbuf_tensor("ones_col", [P, 1], fp16).ap()
    ones_row = nc.alloc_sbuf_tensor("ones_row", [1, P], fp16).ap()

    with tile.TileContext(nc) as tc:
        with (
            tc.tile_pool(name="xs", bufs=2) as xs_pool,
            tc.tile_pool(name="w", bufs=2) as w_pool,
            tc.tile_pool(name="mk", bufs=2) as mk_pool,
            tc.tile_pool(name="ex", bufs=3) as ex_pool,
            tc.tile_pool(name="st", bufs=3) as st_pool,
            tc.tile_pool(name="rb", bufs=2) as rb_pool,
            tc.tile_pool(name="ro", bufs=2) as ro_pool,
            tc.tile_pool(name="ou", bufs=2) as ou_pool,
            tc.tile_pool(name="ps", bufs=2, space="PSUM") as ps_pool,
            tc.tile_pool(name="aux", bufs=4, space="PSUM") as aux_pool,
        ):
            nc.vector.memset(ones_col[:, :], 1.0)
            nc.vector.memset(ones_row[:, :], 1.0)
            nc.sync.dma_start(out=woT_s[:, :, :], in_=woT_t)

            # ---------------- projections ----------------
            for src_t, w_t, dst in (
                (qT_t, wqT_t, qhT),
                (kT_t, wkT_t, khT),
            ):
                w_s = w_pool.tile([P, NMT, DHB], fp16, tag="w")
                nc.sync.dma_start(out=w_s[:, :, :], in_=w_t)
                for ic in range(NIC):
                    x_s = xs_pool.tile([P, NMT, IC], fp16, tag="xs")
                    nc.sync.dma_start(
                        out=x_s[:, :, :],
                        in_=src_t[:, :, ic * IC:(ic + 1) * IC],
                    )
                    for dc in range(2):
                        pp = ps_pool.tile([P, IC], f32, tag="ps")
                        for m in range(NMT):
                            nc.tensor.matmul(
                                pp[:, :],
                                w_s[:, m, dc * P:(dc + 1) * P],
                                x_s[:, m, :],
                                start=(m == 0),
                                stop=(m == NMT - 1),
                            )
                        nc.scalar.copy(
                            dst[:, dc, ic * IC:(ic + 1) * IC], pp[:, :]
                        )

            # v -> vh [128(j), jt, d]
            w_s = w_pool.tile([P, NMT, DHB], fp16, tag="w")
            nc.sync.dma_start(out=w_s[:, :, :], in_=wvT_t)
            for jt in range(NJT):
                x_s = xs_pool.tile([P, NMT, P], fp16, tag="xs")
                nc.sync.dma_start(
                    out=x_s[:, :, :], in_=vT_t[:, :, jt * P:(jt + 1) * P]
                )
                pp = ps_pool.tile([P, DHB], f32, tag="ps")
                for m in range(NMT):
                    nc.tensor.matmul(
                        pp[:, :],
                        x_s[:, m, :],
                        w_s[:, m, :],
                        start=(m == 0),
                        stop=(m == NMT - 1),
                    )
                nc.scalar.copy(vh[:, jt, :], pp[:, :])

            # ---------------- attention, software-pipelined ----------------
            mk = None
            prev = None  # (stripe, rb, h, ic) pending normalize/output/context

            def emit_prev():
                stripe, rb, h, ic = prev
                po = 64 * (h % 2)
                dc = h // 2
                # normalize in place (split DVE / GpSimd), then store + ctx
                nc.vector.tensor_tensor(
                    out=stripe[:, 0:GJT, :],
                    in0=stripe[:, 0:GJT, :],
                    in1=rb[:, None, :].broadcast_to([P, GJT, IC]),
                    op=mybir.AluOpType.mult,
                )
                nc.gpsimd.tensor_tensor(
                    out=stripe[:, GJT:NJT, :],
                    in0=stripe[:, GJT:NJT, :],
                    in1=rb[:, None, :].broadcast_to([P, NJT - GJT, IC]),
                    op=mybir.AluOpType.mult,
                )
                nc.sync.dma_start(
                    out=attnT_t[:, h, :, ic * IC:(ic + 1) * IC],
                    in_=stripe[:, :, :],
                )
                cx = aux_pool.tile([P, IC], f32, tag="aux")
                for jt in range(NJT):
                    nc.tensor.matmul(
                        cx[po:po + 64, :],
                        vh[:, jt, h * DK:(h + 1) * DK],
                        stripe[:, jt, :],
                        start=(jt == 0),
                        stop=(jt == NJT - 1),
                        tile_position=(0, po),
                    )
                nc.scalar.copy(
                    ctxT[po:po + 64, dc, ic * IC:(ic + 1) * IC],
                    cx[po:po + 64, :],
                )

            for ic in range(NIC):
                mk = mk_pool.tile([P, NJT, IC], fp16, tag="mk")
                nc.sync.dma_start(
                    out=mk[:, :, :],
                    in_=maskT_t[:, :, ic * IC:(ic + 1) * IC],
                )
                for h in range(NH):
                    po = 64 * (h % 2)
                    dc = h // 2
                    stripe = st_pool.tile([P, NJT, IC], fp16, tag="st")

                    # S^T + exp + mask, 2 j-tiles at a time
                    for j2 in range(NJT // 2):
                        sp = ps_pool.tile([P, 2, IC], f32, tag="ps")
                        for u in range(2):
                            jt = 2 * j2 + u
                            nc.tensor.matmul(
                                sp[:, u, :],
                                khT[po:po + 64, dc, jt * P:(jt + 1) * P],
                                qhT[po:po + 64, dc, ic * IC:(ic + 1) * IC],
                                start=True,
                                stop=True,
                            )
                        ex = ex_pool.tile([P, 2, IC], fp16, tag="ex")
                        nc.scalar.activation(
                            ex[:, :, :],
                            sp[:, :, :],
                            mybir.ActivationFunctionType.Exp,
                            scale=float(SCALE),
                        )
                        nc.vector.tensor_tensor(
                            out=stripe[:, 2 * j2:2 * j2 + 2, :],
                            in0=ex[:, :, :],
                            in1=mk[:, 2 * j2:2 * j2 + 2, :],
                            op=mybir.AluOpType.mult,
                        )

                    # denominator
                    dn = aux_pool.tile([P, IC], f32, tag="aux")
                    for jt in range(NJT):
                        nc.tensor.matmul(
                            dn[:1, :],
                            ones_col[:, :],
                            stripe[:, jt, :],
                            start=(jt == 0),
                            stop=(jt == NJT - 1),
                        )
                    dr = ro_pool.tile([1, IC], fp16, tag="ro")
                    with nc.allow_low_precision("fp16 softmax denominators"):
                        nc.scalar.copy(dr[:1, :], dn[:1, :])

                    # flush previous step's normalize/store/context
                    if prev is not None:
                        emit_prev()

                    # broadcast + reciprocal -> rb for THIS step
                    rbp = aux_pool.tile([P, IC], f32, tag="aux")
                    nc.tensor.matmul(
                        rbp[:, :], ones_row[:1, :], dr[:1, :],
                        start=True, stop=True,
                    )
                    rb = rb_pool.tile([P, IC], fp16, tag="rb")
                    with nc.allow_low_precision("fp16 softmax reciprocal"):
                        nc.vector.reciprocal(rb[:, :], rbp[:, :])
                    prev = (stripe, rb, h, ic)

            emit_prev()

            # ---------------- output projection ----------------
            for it in range(NJT):
                ob = ou_pool.tile([P, DM], f32, tag="ou")
                for nch in range(2):
                    op = aux_pool.tile([P, IC], f32, tag="aux")
                    for kt in range(2):
                        nc.tensor.matmul(
                            op[:, :],
                            ctxT[:, kt, it * P:(it + 1) * P],
                            woT_s[:, kt, nch * IC:(nch + 1) * IC],
                            start=(kt == 0),
                            stop=(kt == 1),
                        )
                    nc.vector.tensor_copy(
                        ob[:, nch * IC:(nch + 1) * IC], op[:, :]
                    )
                nc.sync.dma_start(out=outp_t[:, it, :], in_=ob[:, :])

    nc.compile()
    return nc


def get_nc():
    if "nc" not in _CACHE:
        _CACHE["nc"] = _build()
    return _CACHE["nc"]


def prep_in_maps(q, k, v, mask, wq, wk, wv, wo_w, wo_b):
    q = np.asarray(q, np.float32)
    k = np.asarray(k, np.float32)
    v = np.asarray(v, np.float32)
    mask = np.asarray(mask)

    maskT_h = [
        np.ascontiguousarray((mask[b_] != 0).T.astype(np.float16))
        for b_ in range(B)
    ]
    qT_h = [np.ascontiguousarray(q[b_].T.astype(np.float16)) for b_ in range(B)]
    kT_h = [np.ascontiguousarray(k[b_].T.astype(np.float16)) for b_ in range(B)]
    vT_h = [np.ascontiguousarray(v[b_].T.astype(np.float16)) for b_ in range(B)]

    in_maps = []
    for c in range(NC_):
        b_ = c // GPB
        g = c % GPB
        ds = slice(DHB * g, DHB * (g + 1))
        in_maps.append({
            "qT": qT_h[b_],
            "kT": kT_h[b_],
            "vT": vT_h[b_],
            "maskT": maskT_h[b_],
            "wqT": np.ascontiguousarray(wq[ds].T.astype(np.float16)),
            "wkT": np.ascontiguousarray(wk[ds].T.astype(np.float16)),
            "wvT": np.ascontiguousarray(wv[ds].T.astype(np.float16)),
            "woT": np.ascontiguousarray(wo_w[:, ds].T.astype(np.float16)),
        })
    return in_maps


def kernel(q, k, v, mask, wq, wk, wv, wo_w, wo_b):
    nc = get_nc()
    wo_b = np.asarray(wo_b, np.float32)
    in_maps = prep_in_maps(q, k, v, mask, wq, wk, wv, wo_w, wo_b)
    _CACHE["last_in_maps"] = in_maps

    res = run_bass_kernel_spmd(nc, in_maps, list(range(NC_)))
    _CACHE["last_results"] = res

    attention = np.empty((B, NHEAD, L, L), np.float32)
    context = np.zeros((B, L, DM), np.float32)
    for c in range(NC_):
        b_ = c // GPB
        g = c % GPB
        at = res.results[c]["attnT"]  # [NH, L(j), L(i)] fp16
        for hl in range(NH):
            attention[b_, NH * g + hl] = at[hl].T.astype(np.float32)
        context[b_] += res.results[c]["outp"]
    context += wo_b[None, None, :]
    return context, attention
